# revision 34
# baseline (speedup 1.0000x reference)
"""LocalMHA (windowed attention, window=128, look_backward=1, RoPE) on 8 TRN2 cores.

Sharding: sequence-parallel, no collectives. Core c handles batch c//2,
sequence half c%2 (2048 query tokens + a 128-token look-backward halo whose
x rows ride along in the core's input shard; zeros at a true sequence start,
where the mask kills the backward keys anyway).

Layout trick: within each 128-row (2-head) block of the head-transposed q/k,
rows are permuted to [hA_d0-31 | hB_d0-31 | hA_d32-63 | hB_d32-63] (host-side
column permutation of W_qkv). The rotate_half partner is then r^64, so RoPE
needs only full-width partition-shifted multiplies, with the sin sign folded
host-side.

RoPE uses GLOBAL angles (f * xs-row); attention scores depend only on the
angle difference, which matches the reference's per-window phases exactly.
k is therefore roped ONCE (not once per window slot) and staged contiguously.
The q scale (dh^-0.5) is folded into W_qkv's q columns host-side.

Softmax: scores are bounded (|s| <~ 8), so exp() runs on the raw PSUM scores;
the band mask is applied as a binary multiply fused with the in-band row-sum
(scalar_tensor_tensor accum), then probs are normalized, transposed on PE,
and av runs in bf16.

Pipelining: attention windows are emitted inside the chunk loop as soon as
their staged q/k/v are available, so the DVE-heavy RoPE phase overlaps the
ACT/DVE-heavy softmax phase. The output projection (PE-heavy, bf16) trails
at the end reading the full bf16 aT tile.

HW pitfalls honored: one matmul accumulation group per PSUM bank for fp32
[128,512] tiles (two groups in one 2KB bank hangs the runtime); PSUM pools
are bank-granular.
"""

import numpy as np
from contextlib import ExitStack

import concourse.bacc as bacc
import concourse.tile as tile
import concourse.mybir as mybir
from concourse.bass_utils import run_bass_kernel_spmd
from concourse.masks import make_identity

# Problem shape (hardcoded per contract)
B, N, D = 4, 4096, 1024
H, DH, WS = 16, 64, 128
THETA = 10000.0
N3 = 3 * H * DH            # 3072
NCORES = 8
HALF = N // 2              # 2048 query tokens per core
NT = HALF + WS             # 2176 tokens incl halo window
NWIN = HALF // WS          # 16 query windows
SCALE = DH ** -0.5

F32 = mybir.dt.float32
F32R = mybir.dt.float32r
BF16 = mybir.dt.bfloat16
ADD = mybir.AluOpType.add
MUL = mybir.AluOpType.mult
EXP = mybir.ActivationFunctionType.Exp

# token chunks for the projection phase (start, len); 128-aligned, len<=512
CHUNKS = [(0, 512), (512, 512), (1024, 512), (1536, 512), (2048, 128)]


def _rope(nc, tmpp, src_psum, dst, L, tab):
    """dst[:, :L] = src*cos + rot64(src)*sin_signed, straight from PSUM.

    Permuted layout: rotate partner of row r is r^64. The sin tile is indexed
    by SOURCE row with the destination's sign folded in host-side. All ops on
    DVE (GPSIMD shares its SBUF port with DVE; offloading there is a loss).
    """
    t1 = tmpp.tile([128, 512], F32, tag="t1")
    nc.vector.tensor_tensor(t1[:, :L], src_psum[:, :L], tab[:, 0, :L], MUL)
    t2 = tmpp.tile([128, 512], F32, tag="t2")
    nc.vector.tensor_tensor(t2[0:64, :L], src_psum[64:128, :L],
                            tab[64:128, 1, :L], MUL)
    nc.vector.tensor_tensor(t2[64:128, :L], src_psum[0:64, :L],
                            tab[0:64, 1, :L], MUL)
    nc.vector.tensor_tensor(dst[:, :L], t1[:, :L], t2[:, :L], ADD)


def _build(reps=1):
    nc = bacc.Bacc("TRN2", target_bir_lowering=False, debug=False,
                   enable_asserts=False, num_devices=NCORES)

    xs = nc.dram_tensor("xs", [NT, D], F32R, kind="ExternalInput").ap()
    wqkv = nc.dram_tensor("wqkv", [D, N3], F32R, kind="ExternalInput").ap()
    wout = nc.dram_tensor("wout", [D, D], BF16, kind="ExternalInput").ap()
    # global-angle rope tables: slot 2*ci = cos, 2*ci+1 = sin for chunk ci;
    # slots 10/11 = chunk-0 q (angles shifted past the halo window)
    ropes = nc.dram_tensor("ropes", [12, 128, 512], F32, kind="ExternalInput").ap()
    masks = nc.dram_tensor("masks", [2, 128, 512], BF16, kind="ExternalInput").ap()
    out = nc.dram_tensor("out", [HALF, D], F32, kind="ExternalOutput").ap()

    # internal DRAM staging
    qrope = nc.dram_tensor("qrope", [D, HALF], F32R).ap()
    kT = nc.dram_tensor("kT", [D, NT], F32R).ap()
    vstage = nc.dram_tensor("vstage", [NT, D], BF16).ap()

    with tile.TileContext(nc) as tc:
        with ExitStack() as top:
            constp = top.enter_context(tc.tile_pool(name="const", bufs=1))
            identf = constp.tile([128, 128], F32, tag="idf")
            make_identity(nc, identf[:])
            identb = constp.tile([128, 128], BF16, tag="idb")
            nc.vector.tensor_copy(identb[:], identf[:])
            identr = constp.tile([128, 128], F32R, tag="idr")
            nc.vector.tensor_copy(identr[:], identf[:])
            mk = constp.tile([128, 2, 512], BF16, tag="masks")
            nc.sync.dma_start(mk[:], masks.rearrange("r p m -> p r m"))
            ropes_r = ropes.rearrange("r p m -> p r m")

            wp = top.enter_context(tc.tile_pool(name="wq", bufs=1))
            w_sb = wp.tile([128, 8, N3], F32R, tag="w")
            wsrc = wqkv.rearrange("(c p) n -> p c n", p=128)

            # attention-phase pools (fresh addresses; open all kernel)
            qwp = top.enter_context(tc.tile_pool(name="qw", bufs=2))
            k2p = top.enter_context(tc.tile_pool(name="k2w", bufs=2))
            vwp = top.enter_context(tc.tile_pool(name="vw", bufs=2))
            ep = top.enter_context(tc.tile_pool(name="e", bufs=2))
            pp = top.enter_context(tc.tile_pool(name="p", bufs=2))
            ptp = top.enter_context(tc.tile_pool(name="pt", bufs=1))
            sump = top.enter_context(tc.tile_pool(name="sums", bufs=4))
            atp = top.enter_context(tc.tile_pool(name="aT", bufs=1))
            aT = atp.tile([128, 8, HALF], BF16, tag="aT")
            sps = top.enter_context(tc.tile_pool(name="sps", bufs=2, space="PSUM"))
            tps2 = top.enter_context(tc.tile_pool(name="tps2", bufs=1, space="PSUM"))
            aps = top.enter_context(tc.tile_pool(name="aps", bufs=1, space="PSUM"))

            UNP = (0, 64, 32, 96)
            vtiles, qw_t, k2_t = {}, {}, {}
            pending = []          # (w, blk) attention pair work queue
            pend_i = 0

            def setup_window(w):
                """Window-level loads for query window w (keys = w-1, w)."""
                qsrc = qrope[:, w * 128:(w + 1) * 128] \
                    .rearrange("(c p) m -> p c m", p=128)
                qw = qwp.tile([128, 8, 128], F32R, tag="qw")
                for g, off in enumerate(UNP):
                    nc.sync.dma_start(qw[g * 32:(g + 1) * 32, :, :],
                                      qsrc[off:off + 32])
                qw_t[w] = qw
                ksrc = kT[:, w * 128: w * 128 + 256] \
                    .rearrange("(c p) j -> p c j", p=128)
                k2w = k2p.tile([128, 8, 256], F32R, tag="k2w")
                for g, off in enumerate(UNP):
                    nc.sync.dma_start(k2w[g * 32:(g + 1) * 32, :, :],
                                      ksrc[off:off + 32])
                k2_t[w] = k2w
                for vt in ([w, w + 1] if w == 0 else [w + 1]):
                    v_t = vwp.tile([128, D], BF16, tag="vw")
                    nc.sync.dma_start(v_t[:], vstage[vt * 128:(vt + 1) * 128, :])
                    vtiles[vt] = v_t

            def emit_pair(w, blk):
                """Attention for head pair (2*blk, 2*blk+1) of window w."""
                if w not in qw_t:
                    setup_window(w)
                qw, k2w = qw_t[w], k2_t[w]
                mslot = 0 if w == 0 else 1
                # scores are bounded (|s|<~8): exp the raw PSUM scores;
                # the mask is a binary multiply fused with the row-sum.
                ee = ep.tile([128, 512], BF16, tag="ee")
                for sub in range(2):
                    spx = sps.tile([128, 256], F32, tag="s")
                    nc.tensor.matmul(spx[:], qw[sub * 64:sub * 64 + 64, blk, :],
                                     k2w[sub * 64:sub * 64 + 64, blk, :],
                                     start=True, stop=True)
                    nc.scalar.activation(ee[:, sub * 256:(sub + 1) * 256],
                                         spx[:], EXP)
                eb = pp.tile([128, 512], BF16, tag="eb")
                ssum = sump.tile([128, 2], F32, tag="ss")
                nc.vector.scalar_tensor_tensor(
                    eb[:, 0:256], ee[:, 0:256], 1.0, mk[:, mslot, 0:256],
                    MUL, MUL, accum_out=ssum[:, 0:1])
                nc.vector.scalar_tensor_tensor(
                    eb[:, 256:512], ee[:, 256:512], 1.0, mk[:, mslot, 256:512],
                    MUL, MUL, accum_out=ssum[:, 1:2])
                rr = sump.tile([128, 2], F32, tag="rr")
                nc.vector.reciprocal(rr[:], ssum[:])
                pf = pp.tile([128, 512], BF16, tag="pf")
                nc.vector.tensor_scalar_mul(pf[:, 0:256], eb[:, 0:256],
                                            rr[:, 0:1])
                nc.vector.tensor_scalar_mul(pf[:, 256:512], eb[:, 256:512],
                                            rr[:, 1:2])
                ptq = tps2.tile([128, 512], BF16, tag="ptq")
                for i in range(4):
                    nc.tensor.transpose(ptq[:, i * 128:(i + 1) * 128],
                                        pf[:, i * 128:(i + 1) * 128],
                                        identb[:])
                pt = ptp.tile([128, 512], BF16, tag="pt")
                nc.scalar.copy(pt[:], ptq[:])
                cA, cB = blk * 128, blk * 128 + 64
                ap_ = aps.tile([64, 256], F32, tag="ap")
                nc.tensor.matmul(ap_[:, 0:128], vtiles[w][:, cA:cA + 64],
                                 pt[:, 0:128], start=True, stop=False)
                nc.tensor.matmul(ap_[:, 0:128], vtiles[w + 1][:, cA:cA + 64],
                                 pt[:, 128:256], start=False, stop=True)
                nc.tensor.matmul(ap_[:, 128:256], vtiles[w][:, cB:cB + 64],
                                 pt[:, 256:384], start=True, stop=False)
                nc.tensor.matmul(ap_[:, 128:256], vtiles[w + 1][:, cB:cB + 64],
                                 pt[:, 384:512], start=False, stop=True)
                nc.vector.tensor_copy(
                    aT[0:64, blk, w * 128:(w + 1) * 128], ap_[:, 0:128])
                nc.vector.tensor_copy(
                    aT[64:128, blk, w * 128:(w + 1) * 128], ap_[:, 128:256])
                if blk == 7:
                    qw_t.pop(w), k2_t.pop(w)
                    vtiles.pop(w - 1, None)

            def drip(n):
                nonlocal pend_i
                stop = min(pend_i + n, len(pending))
                while pend_i < stop:
                    emit_pair(*pending[pend_i])
                    pend_i += 1

            rep_ctx = tc.For_i(0, reps, 1) if reps > 1 else ExitStack()
            top.enter_context(rep_ctx)

            # ---- projection phase (+ attention windows as they unlock) ----
            with ExitStack() as ab:
                tabp = ab.enter_context(tc.tile_pool(name="tab", bufs=2))
                xp = ab.enter_context(tc.tile_pool(name="xst", bufs=2))
                xtp = ab.enter_context(tc.tile_pool(name="xT", bufs=1))
                tmpp = ab.enter_context(tc.tile_pool(name="tmp", bufs=1))
                rop = ab.enter_context(tc.tile_pool(name="ro", bufs=2))
                vp = ab.enter_context(tc.tile_pool(name="vsb", bufs=2))
                tps = ab.enter_context(tc.tile_pool(name="tps", bufs=1, space="PSUM"))
                mps = ab.enter_context(tc.tile_pool(name="mps", bufs=3, space="PSUM"))

                wready = 0
                for ci, (s, L) in enumerate(CHUNKS):
                    nmt = L // 128
                    xT = xtp.tile([128, 8, 512], F32R, tag="xT")
                    for mt in range(nmt):
                        for hf in range(2):
                            xst = xp.tile([128, 512], F32R, tag="x")
                            nc.sync.dma_start(
                                xst[:], xs[s + mt * 128: s + (mt + 1) * 128,
                                           hf * 512:(hf + 1) * 512])
                            for kk in range(4):
                                kc = hf * 4 + kk
                                tp = tps.tile([128, 128], F32R, tag="tp")
                                nc.tensor.transpose(tp[:], xst[:, kk * 128:(kk + 1) * 128],
                                                    identr[:])
                                nc.scalar.copy(xT[:, kc, mt * 128:(mt + 1) * 128], tp[:])
                    if ci == 0:
                        # weights load after chunk-0 x so transposes start at 0
                        for sec in range(3):
                            nc.sync.dma_start(
                                w_sb[:, :, sec * 1024:(sec + 1) * 1024],
                                wsrc[:, :, sec * 1024:(sec + 1) * 1024])

                    tab = tabp.tile([128, 2, 512], F32, tag="tab")
                    nc.sync.dma_start(tab[:], ropes_r[:, 2 * ci:2 * ci + 2, :])
                    qs = 128 if s == 0 else 0
                    qL = L - qs
                    if s == 0:
                        qtab = tabp.tile([128, 2, 512], F32, tag="tab")
                        nc.sync.dma_start(qtab[:], ropes_r[:, 10:12, :])
                    else:
                        qtab = tab
                    if qL > 0:
                        for nch in range(8):
                            qp = mps.tile([128, 512], F32, tag="mm")
                            for kc in range(8):
                                nc.tensor.matmul(qp[:, 0:qL],
                                                 w_sb[:, kc, nch * 128:(nch + 1) * 128],
                                                 xT[:, kc, qs:L],
                                                 start=(kc == 0), stop=(kc == 7))
                            qf = rop.tile([128, 512], F32R, tag="ro")
                            _rope(nc, tmpp, qp, qf, qL, qtab)
                            q0 = s + qs - 128
                            nc.sync.dma_start(
                                qrope[nch * 128:(nch + 1) * 128, q0:q0 + qL],
                                qf[:, 0:qL])

                    # k^T roped once with global angles
                    for nch in range(8):
                        kp = mps.tile([128, 512], F32, tag="mm")
                        for kc in range(8):
                            nc.tensor.matmul(kp[:, :L],
                                             w_sb[:, kc, 1024 + nch * 128: 1024 + (nch + 1) * 128],
                                             xT[:, kc, 0:L],
                                             start=(kc == 0), stop=(kc == 7))
                        kf = rop.tile([128, 512], F32R, tag="ro")
                        _rope(nc, tmpp, kp, kf, L, tab)
                        nc.sync.dma_start(
                            kT[nch * 128:(nch + 1) * 128, s:s + L], kf[:, :L])

                    # v in natural layout, bf16
                    for mt in range(nmt):
                        for nh in range(2):
                            vq = mps.tile([128, 512], F32, tag="mm")
                            for kc in range(8):
                                nc.tensor.matmul(vq[:],
                                                 xT[:, kc, mt * 128:(mt + 1) * 128],
                                                 w_sb[:, kc, 2048 + nh * 512: 2048 + (nh + 1) * 512],
                                                 start=(kc == 0), stop=(kc == 7))
                            vsb = vp.tile([128, 512], BF16, tag="v")
                            nc.scalar.copy(vsb[:], vq[:])
                            nc.sync.dma_start(
                                vstage[s + mt * 128: s + (mt + 1) * 128,
                                       nh * 512:(nh + 1) * 512], vsb[:])

                    # emit attention windows whose inputs are now staged
                    wmax = (s + L) // 128 - 2
                    while wready <= min(wmax, NWIN - 1):
                        emit_c(wready)
                        wready += 1

            # ---------------- output projection (bf16, trails) -------------
            with ExitStack() as dd:
                wop = dd.enter_context(tc.tile_pool(name="wo", bufs=1))
                wo = wop.tile([128, 8, D], BF16, tag="wo")
                nc.sync.dma_start(wo[:], wout.rearrange("(c p) n -> p c n", p=128))
                outp = dd.enter_context(tc.tile_pool(name="outsb", bufs=2))
                ops = dd.enter_context(tc.tile_pool(name="ops", bufs=2, space="PSUM"))
                while wready < NWIN:
                    emit_c(wready)
                    wready += 1
                for mt in range(16):
                    for nh in range(2):
                        op_ = ops.tile([128, 512], F32, tag="op")
                        for kc in range(8):
                            nc.tensor.matmul(op_[:],
                                             aT[:, kc, mt * 128:(mt + 1) * 128],
                                             wo[:, kc, nh * 512:(nh + 1) * 512],
                                             start=(kc == 0), stop=(kc == 7))
                        osb = outp.tile([128, 512], F32, tag="o")
                        nc.scalar.copy(osb[:], op_[:])
                        nc.sync.dma_start(
                            out[mt * 128:(mt + 1) * 128, nh * 512:(nh + 1) * 512],
                            osb[:])

    nc.compile()
    return nc


_NC = {}


def _get_nc(reps=1):
    if reps not in _NC:
        _NC[reps] = _build(reps)
    return _NC[reps]


# permutation within each 128-row (2-head) block of head-transposed q/k:
# new row r holds old row ((r//32)%2)*64 + (r%32) + 32*(r//64)
_r = np.arange(128)
_PERM = ((_r // 32) % 2) * 64 + (_r % 32) + 32 * (_r // 64)


def _host_inputs(x, W_qkv, W_out):
    import ml_dtypes
    # permute q and k column blocks of W_qkv; fold the q scale into W so q
    # and k share one global-angle rope table per chunk
    W = np.ascontiguousarray(W_qkv, np.float32).copy()
    for sec in range(2):                     # q section, k section
        for b in range(8):
            base = sec * 1024 + b * 128
            W[:, base:base + 128] = W[:, base + _PERM]
    W[:, 0:1024] *= SCALE

    invf = THETA ** (-(np.arange(0, 64, 2) / 64.0))          # [32]
    rows_f = invf[_r % 32]                                   # [128] freq per row
    # sin tiles are indexed by SOURCE row of the rotate (partner r^64);
    # the destination sign is folded in per source half.
    rows_s = np.where(_r < 64, 1.0, -1.0)
    # global angle = freq * xs-row index; q at xs row t and key at xs row t
    # use the same angle, so relative phase matches the reference exactly.
    ropes = np.empty((12, 128, 512), np.float32)
    for ci, (s, _L) in enumerate(CHUNKS):
        ang = rows_f[:, None] * (s + np.arange(512))[None, :]
        ropes[2 * ci] = np.cos(ang)
        ropes[2 * ci + 1] = rows_s[:, None] * np.sin(ang)
    ang0 = rows_f[:, None] * (128 + np.arange(512))[None, :]  # chunk0 q
    ropes[10] = np.cos(ang0)
    ropes[11] = rows_s[:, None] * np.sin(ang0)

    i = np.arange(128)[:, None]
    jj = np.arange(256)[None, :]
    band = (jj >= i) & (jj <= i + 128)
    maskB = np.where(band, 1.0, 0.0).astype(np.float32)          # binary mask
    maskA0 = np.where(band & (jj >= 128), 1.0, 0.0).astype(np.float32)
    maskB = np.concatenate([maskB, maskB], axis=1)
    maskA0 = np.concatenate([maskA0, maskA0], axis=1)

    in_maps = []
    for c in range(NCORES):
        bi, hi = c // 2, c % 2
        xsh = np.empty((NT, D), np.float32)
        if hi == 0:
            xsh[:WS] = 0.0
            xsh[WS:] = x[bi, 0:HALF]
            mA = maskA0
        else:
            xsh[:] = x[bi, HALF - WS: N]
            mA = maskB
        in_maps.append({
            "xs": xsh,
            "wqkv": W,
            "wout": np.ascontiguousarray(W_out, np.float32)
                .astype(ml_dtypes.bfloat16),
            "ropes": ropes,
            "masks": np.stack([mA, maskB]).astype(ml_dtypes.bfloat16),
        })
    return in_maps


def kernel(x, W_qkv, W_out):
    x = np.asarray(x, np.float32)
    nc = _get_nc()
    in_maps = _host_inputs(x, W_qkv, W_out)
    res = run_bass_kernel_spmd(nc, in_maps, list(range(NCORES)))
    outf = np.empty((B, N, D), np.float32)
    for c in range(NCORES):
        bi, hi = c // 2, c % 2
        outf[bi, hi * HALF:(hi + 1) * HALF] = res.results[c]["out"]
    return outf


# revision 40
# speedup vs baseline: 1.0152x; 1.0152x over previous
"""LocalMHA (windowed attention, window=128, look_backward=1, RoPE) on 8 TRN2 cores.

Sharding: sequence-parallel, no collectives. Core c handles batch c//2,
sequence half c%2 (2048 query tokens + a 128-token look-backward halo whose
x rows ride along in the core's input shard; zeros at a true sequence start,
where the mask kills the backward keys anyway).

Layout trick: within each 128-row (2-head) block of the head-transposed q/k,
rows are permuted to [hA_d0-31 | hB_d0-31 | hA_d32-63 | hB_d32-63] (host-side
column permutation of W_qkv). The rotate_half partner is then r^64, so RoPE
needs only full-width partition-shifted multiplies, with the sin sign folded
host-side.

RoPE uses GLOBAL angles (f * xs-row); attention scores depend only on the
angle difference, which matches the reference's per-window phases exactly.
k is therefore roped ONCE (not once per window slot) and staged contiguously.
The q scale (dh^-0.5) is folded into W_qkv's q columns host-side.

Softmax: scores are bounded (|s| <~ 8), so exp() runs on the raw PSUM scores;
the band mask is applied as a binary multiply fused with the in-band row-sum
(scalar_tensor_tensor accum), then probs are normalized, transposed on PE,
and av runs in bf16.

Pipelining: attention windows are emitted inside the chunk loop as soon as
their staged q/k/v are available, so the DVE-heavy RoPE phase overlaps the
ACT/DVE-heavy softmax phase. The output projection (PE-heavy, bf16) trails
at the end reading the full bf16 aT tile.

HW pitfalls honored: one matmul accumulation group per PSUM bank for fp32
[128,512] tiles (two groups in one 2KB bank hangs the runtime); PSUM pools
are bank-granular.
"""

import numpy as np
from contextlib import ExitStack

import concourse.bacc as bacc
import concourse.tile as tile
import concourse.mybir as mybir
from concourse.bass_utils import run_bass_kernel_spmd
from concourse.masks import make_identity

# Problem shape (hardcoded per contract)
B, N, D = 4, 4096, 1024
H, DH, WS = 16, 64, 128
THETA = 10000.0
N3 = 3 * H * DH            # 3072
NCORES = 8
HALF = N // 2              # 2048 query tokens per core
NT = HALF + WS             # 2176 tokens incl halo window
NWIN = HALF // WS          # 16 query windows
SCALE = DH ** -0.5

F32 = mybir.dt.float32
F32R = mybir.dt.float32r
BF16 = mybir.dt.bfloat16
ADD = mybir.AluOpType.add
MUL = mybir.AluOpType.mult
EXP = mybir.ActivationFunctionType.Exp

# token chunks for the projection phase (start, len); 128-aligned, len<=512
CHUNKS = [(0, 512), (512, 512), (1024, 512), (1536, 512), (2048, 128)]


def _rope(nc, tmpp, src_psum, dst, L, tab):
    """dst[:, :L] = src*cos + rot64(src)*sin_signed, straight from PSUM.

    Permuted layout: rotate partner of row r is r^64. The sin tile is indexed
    by SOURCE row with the destination's sign folded in host-side. All ops on
    DVE (GPSIMD shares its SBUF port with DVE; offloading there is a loss).
    """
    t1 = tmpp.tile([128, 512], F32, tag="t1")
    nc.vector.tensor_tensor(t1[:, :L], src_psum[:, :L], tab[:, 0, :L], MUL)
    t2 = tmpp.tile([128, 512], F32, tag="t2")
    nc.vector.tensor_tensor(t2[0:64, :L], src_psum[64:128, :L],
                            tab[64:128, 1, :L], MUL)
    nc.vector.tensor_tensor(t2[64:128, :L], src_psum[0:64, :L],
                            tab[0:64, 1, :L], MUL)
    nc.vector.tensor_tensor(dst[:, :L], t1[:, :L], t2[:, :L], ADD)


def _build(reps=1):
    nc = bacc.Bacc("TRN2", target_bir_lowering=False, debug=False,
                   enable_asserts=False, num_devices=NCORES)

    xs = nc.dram_tensor("xs", [NT, D], F32R, kind="ExternalInput").ap()
    wqkv = nc.dram_tensor("wqkv", [D, N3], F32R, kind="ExternalInput").ap()
    wout = nc.dram_tensor("wout", [D, D], BF16, kind="ExternalInput").ap()
    # global-angle rope tables: slot 2*ci = cos, 2*ci+1 = sin for chunk ci;
    # slots 10/11 = chunk-0 q (angles shifted past the halo window)
    ropes = nc.dram_tensor("ropes", [12, 128, 512], F32, kind="ExternalInput").ap()
    masks = nc.dram_tensor("masks", [2, 128, 512], BF16, kind="ExternalInput").ap()
    out = nc.dram_tensor("out", [HALF, D], F32, kind="ExternalOutput").ap()

    # internal DRAM staging
    qrope = nc.dram_tensor("qrope", [D, HALF], F32R).ap()
    kT = nc.dram_tensor("kT", [D, NT], F32R).ap()
    vstage = nc.dram_tensor("vstage", [NT, D], BF16).ap()

    with tile.TileContext(nc) as tc:
        with ExitStack() as top:
            constp = top.enter_context(tc.tile_pool(name="const", bufs=1))
            identf = constp.tile([128, 128], F32, tag="idf")
            make_identity(nc, identf[:])
            identb = constp.tile([128, 128], BF16, tag="idb")
            nc.vector.tensor_copy(identb[:], identf[:])
            identr = constp.tile([128, 128], F32R, tag="idr")
            nc.vector.tensor_copy(identr[:], identf[:])
            mk = constp.tile([128, 2, 512], BF16, tag="masks")
            nc.sync.dma_start(mk[:], masks.rearrange("r p m -> p r m"))
            ropes_r = ropes.rearrange("r p m -> p r m")

            wp = top.enter_context(tc.tile_pool(name="wq", bufs=1))
            w_sb = wp.tile([128, 8, N3], F32R, tag="w")
            wsrc = wqkv.rearrange("(c p) n -> p c n", p=128)

            # attention-phase pools (fresh addresses; open all kernel)
            qwp = top.enter_context(tc.tile_pool(name="qw", bufs=2))
            k2p = top.enter_context(tc.tile_pool(name="k2w", bufs=2))
            vwp = top.enter_context(tc.tile_pool(name="vw", bufs=2))
            ep = top.enter_context(tc.tile_pool(name="e", bufs=2))
            pp = top.enter_context(tc.tile_pool(name="p", bufs=2))
            ptp = top.enter_context(tc.tile_pool(name="pt", bufs=1))
            sump = top.enter_context(tc.tile_pool(name="sums", bufs=4))
            atp = top.enter_context(tc.tile_pool(name="aT", bufs=1))
            aT = atp.tile([128, 8, HALF], BF16, tag="aT")
            sps = top.enter_context(tc.tile_pool(name="sps", bufs=2, space="PSUM"))
            tps2 = top.enter_context(tc.tile_pool(name="tps2", bufs=1, space="PSUM"))
            aps = top.enter_context(tc.tile_pool(name="aps", bufs=1, space="PSUM"))

            UNP = (0, 64, 32, 96)
            vtiles, qw_t, k2_t = {}, {}, {}
            pending = []          # (w, blk) attention pair work queue
            pend_i = 0

            def setup_window(w):
                """Window-level loads for query window w (keys = w-1, w)."""
                qsrc = qrope[:, w * 128:(w + 1) * 128] \
                    .rearrange("(c p) m -> p c m", p=128)
                qw = qwp.tile([128, 8, 128], F32R, tag="qw")
                for g, off in enumerate(UNP):
                    nc.sync.dma_start(qw[g * 32:(g + 1) * 32, :, :],
                                      qsrc[off:off + 32])
                qw_t[w] = qw
                ksrc = kT[:, w * 128: w * 128 + 256] \
                    .rearrange("(c p) j -> p c j", p=128)
                k2w = k2p.tile([128, 8, 256], F32R, tag="k2w")
                for g, off in enumerate(UNP):
                    nc.sync.dma_start(k2w[g * 32:(g + 1) * 32, :, :],
                                      ksrc[off:off + 32])
                k2_t[w] = k2w
                for vt in ([w, w + 1] if w == 0 else [w + 1]):
                    v_t = vwp.tile([128, D], BF16, tag="vw")
                    nc.sync.dma_start(v_t[:], vstage[vt * 128:(vt + 1) * 128, :])
                    vtiles[vt] = v_t

            def emit_pair(w, blk):
                """Attention for head pair (2*blk, 2*blk+1) of window w."""
                if w not in qw_t:
                    setup_window(w)
                qw, k2w = qw_t[w], k2_t[w]
                mslot = 0 if w == 0 else 1
                # scores are bounded (|s|<~8): exp the raw PSUM scores;
                # the mask is a binary multiply fused with the row-sum.
                ee = ep.tile([128, 512], BF16, tag="ee")
                for sub in range(2):
                    spx = sps.tile([128, 256], F32, tag="s")
                    nc.tensor.matmul(spx[:], qw[sub * 64:sub * 64 + 64, blk, :],
                                     k2w[sub * 64:sub * 64 + 64, blk, :],
                                     start=True, stop=True)
                    nc.scalar.activation(ee[:, sub * 256:(sub + 1) * 256],
                                         spx[:], EXP)
                eb = pp.tile([128, 512], BF16, tag="eb")
                ssum = sump.tile([128, 2], F32, tag="ss")
                nc.vector.scalar_tensor_tensor(
                    eb[:, 0:256], ee[:, 0:256], 1.0, mk[:, mslot, 0:256],
                    MUL, MUL, accum_out=ssum[:, 0:1])
                nc.vector.scalar_tensor_tensor(
                    eb[:, 256:512], ee[:, 256:512], 1.0, mk[:, mslot, 256:512],
                    MUL, MUL, accum_out=ssum[:, 1:2])
                rr = sump.tile([128, 2], F32, tag="rr")
                nc.vector.reciprocal(rr[:], ssum[:])
                pf = pp.tile([128, 512], BF16, tag="pf")
                nc.vector.tensor_scalar_mul(pf[:, 0:256], eb[:, 0:256],
                                            rr[:, 0:1])
                nc.vector.tensor_scalar_mul(pf[:, 256:512], eb[:, 256:512],
                                            rr[:, 1:2])
                ptq = tps2.tile([128, 512], BF16, tag="ptq")
                for i in range(4):
                    nc.tensor.transpose(ptq[:, i * 128:(i + 1) * 128],
                                        pf[:, i * 128:(i + 1) * 128],
                                        identb[:])
                pt = ptp.tile([128, 512], BF16, tag="pt")
                nc.scalar.copy(pt[:], ptq[:])
                cA, cB = blk * 128, blk * 128 + 64
                ap_ = aps.tile([64, 256], F32, tag="ap")
                nc.tensor.matmul(ap_[:, 0:128], vtiles[w][:, cA:cA + 64],
                                 pt[:, 0:128], start=True, stop=False)
                nc.tensor.matmul(ap_[:, 0:128], vtiles[w + 1][:, cA:cA + 64],
                                 pt[:, 128:256], start=False, stop=True)
                nc.tensor.matmul(ap_[:, 128:256], vtiles[w][:, cB:cB + 64],
                                 pt[:, 256:384], start=True, stop=False)
                nc.tensor.matmul(ap_[:, 128:256], vtiles[w + 1][:, cB:cB + 64],
                                 pt[:, 384:512], start=False, stop=True)
                nc.vector.tensor_copy(
                    aT[0:64, blk, w * 128:(w + 1) * 128], ap_[:, 0:128])
                nc.vector.tensor_copy(
                    aT[64:128, blk, w * 128:(w + 1) * 128], ap_[:, 128:256])
                if blk == 7:
                    qw_t.pop(w), k2_t.pop(w)
                    vtiles.pop(w - 1, None)

            def drip(n):
                nonlocal pend_i
                stop = min(pend_i + n, len(pending))
                while pend_i < stop:
                    emit_pair(*pending[pend_i])
                    pend_i += 1

            rep_ctx = tc.For_i(0, reps, 1) if reps > 1 else ExitStack()
            top.enter_context(rep_ctx)

            # ---- projection phase (+ attention windows as they unlock) ----
            with ExitStack() as ab:
                tabp = ab.enter_context(tc.tile_pool(name="tab", bufs=2))
                xp = ab.enter_context(tc.tile_pool(name="xst", bufs=2))
                xtp = ab.enter_context(tc.tile_pool(name="xT", bufs=1))
                tmpp = ab.enter_context(tc.tile_pool(name="tmp", bufs=1))
                rop = ab.enter_context(tc.tile_pool(name="ro", bufs=2))
                vp = ab.enter_context(tc.tile_pool(name="vsb", bufs=2))
                tps = ab.enter_context(tc.tile_pool(name="tps", bufs=1, space="PSUM"))
                mps = ab.enter_context(tc.tile_pool(name="mps", bufs=3, space="PSUM"))

                wready = 0
                for ci, (s, L) in enumerate(CHUNKS):
                    nmt = L // 128
                    xT = xtp.tile([128, 8, 512], F32R, tag="xT")
                    for mt in range(nmt):
                        for hf in range(2):
                            xst = xp.tile([128, 512], F32R, tag="x")
                            nc.sync.dma_start(
                                xst[:], xs[s + mt * 128: s + (mt + 1) * 128,
                                           hf * 512:(hf + 1) * 512])
                            for kk in range(4):
                                kc = hf * 4 + kk
                                tp = tps.tile([128, 128], F32R, tag="tp")
                                nc.tensor.transpose(tp[:], xst[:, kk * 128:(kk + 1) * 128],
                                                    identr[:])
                                nc.scalar.copy(xT[:, kc, mt * 128:(mt + 1) * 128], tp[:])
                    if ci == 0:
                        # weights load after chunk-0 x so transposes start at 0
                        for sec in range(3):
                            nc.sync.dma_start(
                                w_sb[:, :, sec * 1024:(sec + 1) * 1024],
                                wsrc[:, :, sec * 1024:(sec + 1) * 1024])

                    tab = tabp.tile([128, 2, 512], F32, tag="tab")
                    nc.sync.dma_start(tab[:], ropes_r[:, 2 * ci:2 * ci + 2, :])
                    qs = 128 if s == 0 else 0
                    qL = L - qs
                    if s == 0:
                        qtab = tabp.tile([128, 2, 512], F32, tag="tab")
                        nc.sync.dma_start(qtab[:], ropes_r[:, 10:12, :])
                    else:
                        qtab = tab
                    if qL > 0:
                        for nch in range(8):
                            qp = mps.tile([128, 512], F32, tag="mm")
                            for kc in range(8):
                                nc.tensor.matmul(qp[:, 0:qL],
                                                 w_sb[:, kc, nch * 128:(nch + 1) * 128],
                                                 xT[:, kc, qs:L],
                                                 start=(kc == 0), stop=(kc == 7))
                            qf = rop.tile([128, 512], F32R, tag="ro")
                            _rope(nc, tmpp, qp, qf, qL, qtab)
                            q0 = s + qs - 128
                            nc.sync.dma_start(
                                qrope[nch * 128:(nch + 1) * 128, q0:q0 + qL],
                                qf[:, 0:qL])
                            drip(2)

                    # k^T roped once with global angles
                    for nch in range(8):
                        kp = mps.tile([128, 512], F32, tag="mm")
                        for kc in range(8):
                            nc.tensor.matmul(kp[:, :L],
                                             w_sb[:, kc, 1024 + nch * 128: 1024 + (nch + 1) * 128],
                                             xT[:, kc, 0:L],
                                             start=(kc == 0), stop=(kc == 7))
                        kf = rop.tile([128, 512], F32R, tag="ro")
                        _rope(nc, tmpp, kp, kf, L, tab)
                        nc.sync.dma_start(
                            kT[nch * 128:(nch + 1) * 128, s:s + L], kf[:, :L])
                        drip(2)

                    # v in natural layout, bf16
                    for mt in range(nmt):
                        for nh in range(2):
                            vq = mps.tile([128, 512], F32, tag="mm")
                            for kc in range(8):
                                nc.tensor.matmul(vq[:],
                                                 xT[:, kc, mt * 128:(mt + 1) * 128],
                                                 w_sb[:, kc, 2048 + nh * 512: 2048 + (nh + 1) * 512],
                                                 start=(kc == 0), stop=(kc == 7))
                            vsb = vp.tile([128, 512], BF16, tag="v")
                            nc.scalar.copy(vsb[:], vq[:])
                            nc.sync.dma_start(
                                vstage[s + mt * 128: s + (mt + 1) * 128,
                                       nh * 512:(nh + 1) * 512], vsb[:])
                            drip(1)

                    # enqueue attention windows whose inputs are now staged
                    wmax = (s + L) // 128 - 2
                    while wready <= min(wmax, NWIN - 1):
                        for blk in range(8):
                            pending.append((wready, blk))
                        wready += 1

            # ---------------- output projection (bf16, trails) -------------
            with ExitStack() as dd:
                wop = dd.enter_context(tc.tile_pool(name="wo", bufs=1))
                wo = wop.tile([128, 8, D], BF16, tag="wo")
                nc.sync.dma_start(wo[:], wout.rearrange("(c p) n -> p c n", p=128))
                outp = dd.enter_context(tc.tile_pool(name="outsb", bufs=2))
                ops = dd.enter_context(tc.tile_pool(name="ops", bufs=2, space="PSUM"))

                def emit_d(mt):
                    for nh in range(2):
                        op_ = ops.tile([128, 512], F32, tag="op")
                        for kc in range(8):
                            nc.tensor.matmul(op_[:],
                                             aT[:, kc, mt * 128:(mt + 1) * 128],
                                             wo[:, kc, nh * 512:(nh + 1) * 512],
                                             start=(kc == 0), stop=(kc == 7))
                        osb = outp.tile([128, 512], F32, tag="o")
                        nc.scalar.copy(osb[:], op_[:])
                        nc.sync.dma_start(
                            out[mt * 128:(mt + 1) * 128, nh * 512:(nh + 1) * 512],
                            osb[:])

                while wready < NWIN:
                    for blk in range(8):
                        pending.append((wready, blk))
                    wready += 1
                mt_done = 0
                while pend_i < len(pending):
                    drip(3)
                    done_w = pend_i // 8    # windows fully emitted
                    while mt_done < min(done_w, NWIN) - 1:
                        emit_d(mt_done)
                        mt_done += 1
                while mt_done < 16:
                    emit_d(mt_done)
                    mt_done += 1

    nc.compile()
    return nc


_NC = {}


def _get_nc(reps=1):
    if reps not in _NC:
        _NC[reps] = _build(reps)
    return _NC[reps]


# permutation within each 128-row (2-head) block of head-transposed q/k:
# new row r holds old row ((r//32)%2)*64 + (r%32) + 32*(r//64)
_r = np.arange(128)
_PERM = ((_r // 32) % 2) * 64 + (_r % 32) + 32 * (_r // 64)


def _host_inputs(x, W_qkv, W_out):
    import ml_dtypes
    # permute q and k column blocks of W_qkv; fold the q scale into W so q
    # and k share one global-angle rope table per chunk
    W = np.ascontiguousarray(W_qkv, np.float32).copy()
    for sec in range(2):                     # q section, k section
        for b in range(8):
            base = sec * 1024 + b * 128
            W[:, base:base + 128] = W[:, base + _PERM]
    W[:, 0:1024] *= SCALE

    invf = THETA ** (-(np.arange(0, 64, 2) / 64.0))          # [32]
    rows_f = invf[_r % 32]                                   # [128] freq per row
    # sin tiles are indexed by SOURCE row of the rotate (partner r^64);
    # the destination sign is folded in per source half.
    rows_s = np.where(_r < 64, 1.0, -1.0)
    # global angle = freq * xs-row index; q at xs row t and key at xs row t
    # use the same angle, so relative phase matches the reference exactly.
    ropes = np.empty((12, 128, 512), np.float32)
    for ci, (s, _L) in enumerate(CHUNKS):
        ang = rows_f[:, None] * (s + np.arange(512))[None, :]
        ropes[2 * ci] = np.cos(ang)
        ropes[2 * ci + 1] = rows_s[:, None] * np.sin(ang)
    ang0 = rows_f[:, None] * (128 + np.arange(512))[None, :]  # chunk0 q
    ropes[10] = np.cos(ang0)
    ropes[11] = rows_s[:, None] * np.sin(ang0)

    i = np.arange(128)[:, None]
    jj = np.arange(256)[None, :]
    band = (jj >= i) & (jj <= i + 128)
    maskB = np.where(band, 1.0, 0.0).astype(np.float32)          # binary mask
    maskA0 = np.where(band & (jj >= 128), 1.0, 0.0).astype(np.float32)
    maskB = np.concatenate([maskB, maskB], axis=1)
    maskA0 = np.concatenate([maskA0, maskA0], axis=1)

    in_maps = []
    for c in range(NCORES):
        bi, hi = c // 2, c % 2
        xsh = np.empty((NT, D), np.float32)
        if hi == 0:
            xsh[:WS] = 0.0
            xsh[WS:] = x[bi, 0:HALF]
            mA = maskA0
        else:
            xsh[:] = x[bi, HALF - WS: N]
            mA = maskB
        in_maps.append({
            "xs": xsh,
            "wqkv": W,
            "wout": np.ascontiguousarray(W_out, np.float32)
                .astype(ml_dtypes.bfloat16),
            "ropes": ropes,
            "masks": np.stack([mA, maskB]).astype(ml_dtypes.bfloat16),
        })
    return in_maps


def kernel(x, W_qkv, W_out):
    x = np.asarray(x, np.float32)
    nc = _get_nc()
    in_maps = _host_inputs(x, W_qkv, W_out)
    res = run_bass_kernel_spmd(nc, in_maps, list(range(NCORES)))
    outf = np.empty((B, N, D), np.float32)
    for c in range(NCORES):
        bi, hi = c // 2, c % 2
        outf[bi, hi * HALF:(hi + 1) * HALF] = res.results[c]["out"]
    return outf


# revision 41
# speedup vs baseline: 1.1148x; 1.0980x over previous
"""LocalMHA (windowed attention, window=128, look_backward=1, RoPE) on 8 TRN2 cores.

Sharding: sequence-parallel, no collectives. Core c handles batch c//2,
sequence half c%2 (2048 query tokens + a 128-token look-backward halo whose
x rows ride along in the core's input shard; zeros at a true sequence start,
where the mask kills the backward keys anyway).

Layout trick: within each 128-row (2-head) block of the head-transposed q/k,
rows are permuted to [hA_d0-31 | hB_d0-31 | hA_d32-63 | hB_d32-63] (host-side
column permutation of W_qkv). The rotate_half partner is then r^64, so RoPE
needs only full-width partition-shifted multiplies (no 32-row fragments), with
the sin sign folded host-side. Scores contract each head's d over two 32-row
chunks (PSUM accumulation) — the dot product is invariant to the d-permutation.

Dtypes: projections and scores in fp32r (full PE rate at free>=256, ~1.6e-4);
attention probabilities and v in bf16 (free-dim-128 matmuls at full rate).

Engine split: PE matmuls/transposes; DVE elementwise (RoPE, mask-add,
normalize, most evictions); ACT exp(+fused row-sum) and the remaining psum
evictions. GPSIMD is left idle on purpose — it shares its SBUF port with DVE
under an exclusive lock, so "offloading" to it stalls DVE (measured +35%).
"""

import numpy as np
from contextlib import ExitStack

import concourse.bacc as bacc
import concourse.tile as tile
import concourse.mybir as mybir
from concourse.bass_utils import run_bass_kernel_spmd
from concourse.masks import make_identity

# Problem shape (hardcoded per contract)
B, N, D = 4, 4096, 1024
H, DH, WS = 16, 64, 128
THETA = 10000.0
N3 = 3 * H * DH            # 3072
NCORES = 8
HALF = N // 2              # 2048 query tokens per core
NT = HALF + WS             # 2176 tokens incl halo window
NWIN = HALF // WS          # 16 query windows
SCALE = DH ** -0.5
NEG = -1.0e9

F32 = mybir.dt.float32
F32R = mybir.dt.float32r
BF16 = mybir.dt.bfloat16
ADD = mybir.AluOpType.add
MUL = mybir.AluOpType.mult
EXP = mybir.ActivationFunctionType.Exp

# token chunks for phases A/B (start, len); 128-aligned, len<=512
CHUNKS = [(0, 512), (512, 512), (1024, 512), (1536, 512), (2048, 128)]


def _rope(nc, tmpp, src_psum, dst, s0, L, rp, ci, si):
    """dst[:, s0:L] = src*cos + rot64(src)*sin_signed on cols [s0, L).

    Permuted layout: rotate partner of row r is r^64. The sin tile is indexed
    by SOURCE row with the destination's sign folded in host-side, so each
    multiply's two inputs share a base partition (only outputs are shifted —
    the ISA allows that). All ops stay on DVE: GPSIMD shares its SBUF port
    with DVE under an exclusive lock, so offloading there is a net loss.
    """
    t1 = tmpp.tile([128, 512], F32, tag="t1")
    nc.vector.tensor_tensor(t1[:, s0:L], src_psum[:, s0:L], rp[:, ci, s0:L], MUL)
    t2 = tmpp.tile([128, 512], F32, tag="t2")
    nc.vector.tensor_tensor(t2[0:64, s0:L], src_psum[64:128, s0:L],
                            rp[64:128, si, s0:L], MUL)
    nc.vector.tensor_tensor(t2[64:128, s0:L], src_psum[0:64, s0:L],
                            rp[0:64, si, s0:L], MUL)
    nc.vector.tensor_tensor(dst[:, s0:L], t1[:, s0:L], t2[:, s0:L], ADD)


def _build(reps=1):
    nc = bacc.Bacc("TRN2", target_bir_lowering=False, debug=False,
                   enable_asserts=False, num_devices=NCORES)

    xs = nc.dram_tensor("xs", [NT, D], F32R, kind="ExternalInput").ap()
    wqkv = nc.dram_tensor("wqkv", [D, N3], F32R, kind="ExternalInput").ap()
    wout = nc.dram_tensor("wout", [D, D], F32R, kind="ExternalInput").ap()
    # global-angle rope tables: slot 2*ci = cos, 2*ci+1 = sin for chunk ci;
    # slots 10/11 = chunk-0 q (angles shifted by the halo window)
    ropes = nc.dram_tensor("ropes", [12, 128, 512], F32, kind="ExternalInput").ap()
    masks = nc.dram_tensor("masks", [2, 128, 512], F32, kind="ExternalInput").ap()
    out = nc.dram_tensor("out", [HALF, D], F32, kind="ExternalOutput").ap()

    # internal DRAM staging
    qrope = nc.dram_tensor("qrope", [D, HALF], F32R).ap()
    kT = nc.dram_tensor("kT", [D, NT], F32R).ap()
    vstage = nc.dram_tensor("vstage", [NT, D], BF16).ap()

    with tile.TileContext(nc) as tc:
        with ExitStack() as top:
            constp = top.enter_context(tc.tile_pool(name="const", bufs=1))
            identf = constp.tile([128, 128], F32, tag="idf")
            make_identity(nc, identf[:])
            identb = constp.tile([128, 128], BF16, tag="idb")
            nc.vector.tensor_copy(identb[:], identf[:])
            identr = constp.tile([128, 128], F32R, tag="idr")
            nc.vector.tensor_copy(identr[:], identf[:])
            rp = constp.tile([128, 12, 512], F32, tag="ropes")
            nc.sync.dma_start(rp[:], ropes.rearrange("r p m -> p r m"))
            mk = constp.tile([128, 2, 512], F32, tag="masks")
            nc.sync.dma_start(mk[:], masks.rearrange("r p m -> p r m"))

            rep_ctx = tc.For_i(0, reps, 1) if reps > 1 else ExitStack()
            top.enter_context(rep_ctx)

            # ---------------- Phase A+B: transpose + QKV + RoPE ----------------
            with ExitStack() as ab:
                wp = ab.enter_context(tc.tile_pool(name="wq", bufs=1))
                w_sb = wp.tile([128, 8, N3], F32R, tag="w")
                wsrc = wqkv.rearrange("(c p) n -> p c n", p=128)
                xp = ab.enter_context(tc.tile_pool(name="xst", bufs=2))
                xtp = ab.enter_context(tc.tile_pool(name="xT", bufs=2))
                tmpp = ab.enter_context(tc.tile_pool(name="tmp", bufs=3))
                rop = ab.enter_context(tc.tile_pool(name="ro", bufs=4))
                vp = ab.enter_context(tc.tile_pool(name="vsb", bufs=3))
                tps = ab.enter_context(tc.tile_pool(name="tps", bufs=3, space="PSUM"))
                mps = ab.enter_context(tc.tile_pool(name="mps", bufs=5, space="PSUM"))

                for ci, (s, L) in enumerate(CHUNKS):
                    nmt = L // 128
                    xT = xtp.tile([128, 8, 512], F32R, tag="xT")
                    for mt in range(nmt):
                        xst = xp.tile([128, D], F32R, tag="x")
                        nc.sync.dma_start(xst[:], xs[s + mt * 128: s + (mt + 1) * 128, :])
                        for kc in range(8):
                            tp = tps.tile([128, 128], F32R, tag="tp")
                            nc.tensor.transpose(tp[:], xst[:, kc * 128:(kc + 1) * 128],
                                                identr[:])
                            nc.scalar.copy(xT[:, kc, mt * 128:(mt + 1) * 128], tp[:])
                    if ci == 0:
                        # weights load behind chunk-0 x so transposes start at 0
                        for sec in range(3):
                            nc.sync.dma_start(
                                w_sb[:, :, sec * 1024:(sec + 1) * 1024],
                                wsrc[:, :, sec * 1024:(sec + 1) * 1024])

                    ct = 2 * ci  # cos table slot; sin = ct+1 (q scale folded in W)
                    # q^T (+rope) for query tokens of this chunk
                    qs = 128 if s == 0 else 0
                    qL = L - qs
                    qt = 10 if s == 0 else ct  # chunk0 q uses shifted table
                    if qL > 0:
                        for nch in range(8):
                            qp = mps.tile([128, 512], F32, tag="mm")
                            for kc in range(8):
                                nc.tensor.matmul(qp[:, 0:qL],
                                                 w_sb[:, kc, nch * 128:(nch + 1) * 128],
                                                 xT[:, kc, qs:L],
                                                 start=(kc == 0), stop=(kc == 7))
                            qf = rop.tile([128, 512], F32R, tag="ro")
                            _rope(nc, tmpp, qp, qf, 0, qL, rp, qt, qt + 1)
                            q0 = s + qs - 128
                            nc.sync.dma_start(
                                qrope[nch * 128:(nch + 1) * 128, q0:q0 + qL],
                                qf[:, 0:qL])

                    # k^T roped once with global angles
                    for nch in range(8):
                        kp = mps.tile([128, 512], F32, tag="mm")
                        for kc in range(8):
                            nc.tensor.matmul(kp[:, :L],
                                             w_sb[:, kc, 1024 + nch * 128: 1024 + (nch + 1) * 128],
                                             xT[:, kc, 0:L],
                                             start=(kc == 0), stop=(kc == 7))
                        kf = rop.tile([128, 512], F32R, tag="ro")
                        _rope(nc, tmpp, kp, kf, 0, L, rp, ct, ct + 1)
                        nc.sync.dma_start(
                            kT[nch * 128:(nch + 1) * 128, s:s + L], kf[:, :L])

                    # v in natural layout, bf16
                    for mt in range(nmt):
                        vsb = vp.tile([128, D], BF16, tag="v")
                        for nh in range(2):
                            vq = mps.tile([128, 512], F32, tag="mm")
                            for kc in range(8):
                                nc.tensor.matmul(vq[:],
                                                 xT[:, kc, mt * 128:(mt + 1) * 128],
                                                 w_sb[:, kc, 2048 + nh * 512: 2048 + (nh + 1) * 512],
                                                 start=(kc == 0), stop=(kc == 7))
                            nc.scalar.copy(vsb[:, nh * 512:(nh + 1) * 512], vq[:])
                        nc.sync.dma_start(vstage[s + mt * 128: s + (mt + 1) * 128, :], vsb[:])

            # ---------------- Phase C: windowed attention ----------------
            with ExitStack() as cd:
                atp = cd.enter_context(tc.tile_pool(name="aT", bufs=1))
                aT = atp.tile([128, 8, HALF], F32R, tag="aT")
                if True:
                    cc = cd
                    qwp = cc.enter_context(tc.tile_pool(name="qw", bufs=3))
                    k2p = cc.enter_context(tc.tile_pool(name="k2w", bufs=3))
                    vwp = cc.enter_context(tc.tile_pool(name="vw", bufs=4))
                    ep = cc.enter_context(tc.tile_pool(name="e", bufs=4))
                    pp = cc.enter_context(tc.tile_pool(name="p", bufs=3))
                    ptp = cc.enter_context(tc.tile_pool(name="pt", bufs=3))
                    sump = cc.enter_context(tc.tile_pool(name="sums", bufs=4))
                    sps = cc.enter_context(tc.tile_pool(name="sps", bufs=2, space="PSUM"))
                    tps2 = cc.enter_context(tc.tile_pool(name="tps2", bufs=2, space="PSUM"))
                    aps = cc.enter_context(tc.tile_pool(name="aps", bufs=2, space="PSUM"))
                    # phase D pools live alongside C so D(w-1) interleaves
                    # with C(w): D is PE-heavy, C is DVE/ACT-heavy.
                    wop = cc.enter_context(tc.tile_pool(name="wo", bufs=1))
                    wo = wop.tile([128, 8, D], F32R, tag="wo")
                    nc.sync.dma_start(wo[:], wout.rearrange("(c p) n -> p c n", p=128))
                    outp = cc.enter_context(tc.tile_pool(name="outsb", bufs=2))
                    ops = cc.enter_context(tc.tile_pool(name="ops", bufs=2, space="PSUM"))

                    def emit_d(mt):
                        osb = outp.tile([128, D], F32, tag="o")
                        for nh in range(2):
                            op_ = ops.tile([128, 512], F32, tag="op")
                            for kc in range(8):
                                nc.tensor.matmul(op_[:],
                                                 aT[:, kc, mt * 128:(mt + 1) * 128],
                                                 wo[:, kc, nh * 512:(nh + 1) * 512],
                                                 start=(kc == 0), stop=(kc == 7))
                            nc.scalar.copy(osb[:, nh * 512:(nh + 1) * 512], op_[:])
                        nc.sync.dma_start(out[mt * 128:(mt + 1) * 128, :], osb[:])

                    # un-permute staged q/k on load: target row groups
                    # [hA_lo, hA_hi, hB_lo, hB_hi] <- permuted-source offsets
                    UNP = (0, 64, 32, 96)
                    vtiles = {}
                    for w in range(NWIN):
                        qsrc = qrope[:, w * 128:(w + 1) * 128] \
                            .rearrange("(c p) m -> p c m", p=128)
                        qw = qwp.tile([128, 8, 128], F32R, tag="qw")
                        for g, off in enumerate(UNP):
                            nc.sync.dma_start(qw[g * 32:(g + 1) * 32, :, :],
                                              qsrc[off:off + 32])
                        ksrc = kT[:, w * 128: w * 128 + 256] \
                            .rearrange("(c p) j -> p c j", p=128)
                        k2w = k2p.tile([128, 8, 256], F32R, tag="k2w")
                        for g, off in enumerate(UNP):
                            nc.sync.dma_start(k2w[g * 32:(g + 1) * 32, :, :],
                                              ksrc[off:off + 32])
                        for vt in ([w, w + 1] if w == 0 else [w + 1]):
                            v_t = vwp.tile([128, D], BF16, tag="vw")
                            nc.sync.dma_start(v_t[:], vstage[vt * 128:(vt + 1) * 128, :])
                            vtiles[vt] = v_t
                        mslot = 0 if w == 0 else 1
                        for blk in range(8):          # head pair 2*blk, 2*blk+1
                            # scores are bounded (|s|<~8) so exp the raw PSUM
                            # scores; the mask is applied as a binary multiply
                            # fused with the in-band row-sum below.
                            ee = ep.tile([128, 512], F32, tag="ee")
                            for sub in range(2):
                                spx = sps.tile([128, 256], F32, tag="s")
                                nc.tensor.matmul(spx[:], qw[sub * 64:sub * 64 + 64, blk, :],
                                                 k2w[sub * 64:sub * 64 + 64, blk, :],
                                                 start=True, stop=True)
                                nc.scalar.activation(ee[:, sub * 256:(sub + 1) * 256],
                                                     spx[:], EXP)
                            eb = pp.tile([128, 512], BF16, tag="eb")
                            ssum = sump.tile([128, 2], F32, tag="ss")
                            nc.vector.scalar_tensor_tensor(
                                eb[:, 0:256], ee[:, 0:256], 1.0, mk[:, mslot, 0:256],
                                MUL, MUL, accum_out=ssum[:, 0:1])
                            nc.vector.scalar_tensor_tensor(
                                eb[:, 256:512], ee[:, 256:512], 1.0,
                                mk[:, mslot, 256:512],
                                MUL, MUL, accum_out=ssum[:, 1:2])
                            rr = sump.tile([128, 2], F32, tag="rr")
                            nc.vector.reciprocal(rr[:], ssum[:])
                            pf = pp.tile([128, 512], BF16, tag="pf")
                            nc.vector.tensor_scalar_mul(pf[:, 0:256], eb[:, 0:256],
                                                        rr[:, 0:1])
                            nc.vector.tensor_scalar_mul(pf[:, 256:512], eb[:, 256:512],
                                                        rr[:, 1:2])
                            ptq = tps2.tile([128, 512], BF16, tag="ptq")
                            for i in range(4):
                                nc.tensor.transpose(ptq[:, i * 128:(i + 1) * 128],
                                                    pf[:, i * 128:(i + 1) * 128],
                                                    identb[:])
                            pt = ptp.tile([128, 512], BF16, tag="pt")
                            nc.scalar.copy(pt[:], ptq[:])
                            cA, cB = blk * 128, blk * 128 + 64
                            ap_ = aps.tile([64, 256], F32, tag="ap")
                            nc.tensor.matmul(ap_[:, 0:128], vtiles[w][:, cA:cA + 64],
                                             pt[:, 0:128], start=True, stop=False)
                            nc.tensor.matmul(ap_[:, 0:128], vtiles[w + 1][:, cA:cA + 64],
                                             pt[:, 128:256], start=False, stop=True)
                            nc.tensor.matmul(ap_[:, 128:256], vtiles[w][:, cB:cB + 64],
                                             pt[:, 256:384], start=True, stop=False)
                            nc.tensor.matmul(ap_[:, 128:256], vtiles[w + 1][:, cB:cB + 64],
                                             pt[:, 384:512], start=False, stop=True)
                            nc.vector.tensor_copy(
                                aT[0:64, blk, w * 128:(w + 1) * 128], ap_[:, 0:128])
                            nc.vector.tensor_copy(
                                aT[64:128, blk, w * 128:(w + 1) * 128], ap_[:, 128:256])
                        vtiles.pop(w - 1, None)
                    for mt in range(16):
                        emit_d(mt)

    nc.compile()
    return nc


_NC = {}


def _get_nc(reps=1):
    if reps not in _NC:
        _NC[reps] = _build(reps)
    return _NC[reps]


# permutation within each 128-row (2-head) block of head-transposed q/k:
# new row r holds old row ((r//32)%2)*64 + (r%32) + 32*(r//64)
_r = np.arange(128)
_PERM = ((_r // 32) % 2) * 64 + (_r % 32) + 32 * (_r // 64)


def _host_inputs(x, W_qkv, W_out):
    # permute q and k column blocks of W_qkv; fold the q scale into W so q
    # and k share one global-angle rope table per chunk
    W = np.ascontiguousarray(W_qkv, np.float32).copy()
    for sec in range(2):                     # q section, k section
        for b in range(8):
            base = sec * 1024 + b * 128
            W[:, base:base + 128] = W[:, base + _PERM]
    W[:, 0:1024] *= SCALE

    invf = THETA ** (-(np.arange(0, 64, 2) / 64.0))          # [32]
    rows_f = invf[_r % 32]                                   # [128] freq per row
    # sin tiles are indexed by SOURCE row of the rotate (partner r^64);
    # the destination sign is +1 when the source is a hi-half (r>=64).
    rows_s = np.where(_r < 64, 1.0, -1.0)
    # global angle = freq * xs-row index; q at xs row t and key at xs row t
    # use the same angle, so relative phase matches the reference exactly.
    ropes = np.empty((12, 128, 512), np.float32)
    for ci, (s, _L) in enumerate(CHUNKS):
        ang = rows_f[:, None] * (s + np.arange(512))[None, :]
        ropes[2 * ci] = np.cos(ang)
        ropes[2 * ci + 1] = rows_s[:, None] * np.sin(ang)
    ang0 = rows_f[:, None] * (128 + np.arange(512))[None, :]  # chunk0 q
    ropes[10] = np.cos(ang0)
    ropes[11] = rows_s[:, None] * np.sin(ang0)

    i = np.arange(128)[:, None]
    jj = np.arange(256)[None, :]
    band = (jj >= i) & (jj <= i + 128)
    maskB = np.where(band, 1.0, 0.0).astype(np.float32)          # binary mask
    maskA0 = np.where(band & (jj >= 128), 1.0, 0.0).astype(np.float32)
    maskB = np.concatenate([maskB, maskB], axis=1)
    maskA0 = np.concatenate([maskA0, maskA0], axis=1)

    in_maps = []
    for c in range(NCORES):
        bi, hi = c // 2, c % 2
        xsh = np.empty((NT, D), np.float32)
        if hi == 0:
            xsh[:WS] = 0.0
            xsh[WS:] = x[bi, 0:HALF]
            mA = maskA0
        else:
            xsh[:] = x[bi, HALF - WS: N]
            mA = maskB
        in_maps.append({
            "xs": xsh,
            "wqkv": W,
            "wout": np.ascontiguousarray(W_out, np.float32),
            "ropes": ropes,
            "masks": np.stack([mA, maskB]),
        })
    return in_maps


def kernel(x, W_qkv, W_out):
    x = np.asarray(x, np.float32)
    nc = _get_nc()
    in_maps = _host_inputs(x, W_qkv, W_out)
    res = run_bass_kernel_spmd(nc, in_maps, list(range(NCORES)))
    outf = np.empty((B, N, D), np.float32)
    for c in range(NCORES):
        bi, hi = c // 2, c % 2
        outf[bi, hi * HALF:(hi + 1) * HALF] = res.results[c]["out"]
    return outf



# revision 43
# speedup vs baseline: 1.1557x; 1.0367x over previous
"""LocalMHA (windowed attention, window=128, look_backward=1, RoPE) on 8 TRN2 cores.

Sharding: sequence-parallel, no collectives. Core c handles batch c//2,
sequence half c%2 (2048 query tokens + a 128-token look-backward halo whose
x rows ride along in the core's input shard; zeros at a true sequence start,
where the mask kills the backward keys anyway).

Layout trick: within each 128-row (2-head) block of the head-transposed q/k,
rows are permuted to [hA_d0-31 | hB_d0-31 | hA_d32-63 | hB_d32-63] (host-side
column permutation of W_qkv). The rotate_half partner is then r^64, so RoPE
needs only full-width partition-shifted multiplies (no 32-row fragments), with
the sin sign folded host-side. Scores contract each head's d over two 32-row
chunks (PSUM accumulation) — the dot product is invariant to the d-permutation.

Dtypes: projections and scores in fp32r (full PE rate at free>=256, ~1.6e-4);
attention probabilities and v in bf16 (free-dim-128 matmuls at full rate).

Engine split: PE matmuls/transposes; DVE elementwise (RoPE, mask-add,
normalize, most evictions); ACT exp(+fused row-sum) and the remaining psum
evictions. GPSIMD is left idle on purpose — it shares its SBUF port with DVE
under an exclusive lock, so "offloading" to it stalls DVE (measured +35%).
"""

import numpy as np
from contextlib import ExitStack

import concourse.bacc as bacc
import concourse.tile as tile
import concourse.mybir as mybir
from concourse.bass_utils import run_bass_kernel_spmd
from concourse.masks import make_identity

# Problem shape (hardcoded per contract)
B, N, D = 4, 4096, 1024
H, DH, WS = 16, 64, 128
THETA = 10000.0
N3 = 3 * H * DH            # 3072
NCORES = 8
HALF = N // 2              # 2048 query tokens per core
NT = HALF + WS             # 2176 tokens incl halo window
NWIN = HALF // WS          # 16 query windows
SCALE = DH ** -0.5
NEG = -1.0e9

F32 = mybir.dt.float32
F32R = mybir.dt.float32r
BF16 = mybir.dt.bfloat16
ADD = mybir.AluOpType.add
MUL = mybir.AluOpType.mult
EXP = mybir.ActivationFunctionType.Exp

# token chunks for phases A/B (start, len); 128-aligned, len<=512
CHUNKS = [(0, 512), (512, 512), (1024, 512), (1536, 512), (2048, 128)]


def _rope(nc, tmpp, src_psum, dst, s0, L, rp, ci, si):
    """dst[:, s0:L] = src*cos + rot64(src)*sin_signed on cols [s0, L).

    Permuted layout: rotate partner of row r is r^64. The sin tile is indexed
    by SOURCE row with the destination's sign folded in host-side, so each
    multiply's two inputs share a base partition (only outputs are shifted —
    the ISA allows that). All ops stay on DVE: GPSIMD shares its SBUF port
    with DVE under an exclusive lock, so offloading there is a net loss.
    """
    t1 = tmpp.tile([128, 512], F32, tag="t1")
    nc.vector.tensor_tensor(t1[:, s0:L], src_psum[:, s0:L], rp[:, ci, s0:L], MUL)
    t2 = tmpp.tile([128, 512], F32, tag="t2")
    nc.vector.tensor_tensor(t2[0:64, s0:L], src_psum[64:128, s0:L],
                            rp[64:128, si, s0:L], MUL)
    nc.vector.tensor_tensor(t2[64:128, s0:L], src_psum[0:64, s0:L],
                            rp[0:64, si, s0:L], MUL)
    nc.vector.tensor_tensor(dst[:, s0:L], t1[:, s0:L], t2[:, s0:L], ADD)


def _build(reps=1):
    nc = bacc.Bacc("TRN2", target_bir_lowering=False, debug=False,
                   enable_asserts=False, num_devices=NCORES)

    xs = nc.dram_tensor("xs", [NT, D], F32R, kind="ExternalInput").ap()
    wqkv = nc.dram_tensor("wqkv", [D, N3], F32R, kind="ExternalInput").ap()
    wout = nc.dram_tensor("wout", [D, D], BF16, kind="ExternalInput").ap()
    # global-angle rope tables: slot 2*ci = cos, 2*ci+1 = sin for chunk ci;
    # slots 10/11 = chunk-0 q (angles shifted by the halo window)
    ropes = nc.dram_tensor("ropes", [12, 128, 512], F32, kind="ExternalInput").ap()
    masks = nc.dram_tensor("masks", [2, 128, 512], F32, kind="ExternalInput").ap()
    out = nc.dram_tensor("out", [HALF, D], F32, kind="ExternalOutput").ap()

    # internal DRAM staging
    qrope = nc.dram_tensor("qrope", [D, HALF], F32R).ap()
    kT = nc.dram_tensor("kT", [D, NT], F32R).ap()
    vstage = nc.dram_tensor("vstage", [NT, D], BF16).ap()

    with tile.TileContext(nc) as tc:
        with ExitStack() as top:
            constp = top.enter_context(tc.tile_pool(name="const", bufs=1))
            identf = constp.tile([128, 128], F32, tag="idf")
            make_identity(nc, identf[:])
            identb = constp.tile([128, 128], BF16, tag="idb")
            nc.vector.tensor_copy(identb[:], identf[:])
            identr = constp.tile([128, 128], F32R, tag="idr")
            nc.vector.tensor_copy(identr[:], identf[:])
            rp = constp.tile([128, 12, 512], F32, tag="ropes")
            nc.sync.dma_start(rp[:], ropes.rearrange("r p m -> p r m"))
            mk = constp.tile([128, 2, 512], F32, tag="masks")
            nc.sync.dma_start(mk[:], masks.rearrange("r p m -> p r m"))

            rep_ctx = tc.For_i(0, reps, 1) if reps > 1 else ExitStack()
            top.enter_context(rep_ctx)

            # ---------------- Phase A+B: transpose + QKV + RoPE ----------------
            with ExitStack() as ab:
                wp = ab.enter_context(tc.tile_pool(name="wq", bufs=1))
                w_sb = wp.tile([128, 8, N3], F32R, tag="w")
                wsrc = wqkv.rearrange("(c p) n -> p c n", p=128)
                xp = ab.enter_context(tc.tile_pool(name="xst", bufs=2))
                xtp = ab.enter_context(tc.tile_pool(name="xT", bufs=2))
                tmpp = ab.enter_context(tc.tile_pool(name="tmp", bufs=3))
                rop = ab.enter_context(tc.tile_pool(name="ro", bufs=4))
                vp = ab.enter_context(tc.tile_pool(name="vsb", bufs=3))
                tps = ab.enter_context(tc.tile_pool(name="tps", bufs=3, space="PSUM"))
                mps = ab.enter_context(tc.tile_pool(name="mps", bufs=5, space="PSUM"))

                for ci, (s, L) in enumerate(CHUNKS):
                    nmt = L // 128
                    xT = xtp.tile([128, 8, 512], F32R, tag="xT")
                    for mt in range(nmt):
                        xst = xp.tile([128, D], F32R, tag="x")
                        nc.sync.dma_start(xst[:], xs[s + mt * 128: s + (mt + 1) * 128, :])
                        for kc in range(8):
                            tp = tps.tile([128, 128], F32R, tag="tp")
                            nc.tensor.transpose(tp[:], xst[:, kc * 128:(kc + 1) * 128],
                                                identr[:])
                            nc.scalar.copy(xT[:, kc, mt * 128:(mt + 1) * 128], tp[:])
                    if ci == 0:
                        # weights load behind chunk-0 x so transposes start at 0
                        for sec in range(3):
                            nc.sync.dma_start(
                                w_sb[:, :, sec * 1024:(sec + 1) * 1024],
                                wsrc[:, :, sec * 1024:(sec + 1) * 1024])

                    ct = 2 * ci  # cos table slot; sin = ct+1 (q scale folded in W)
                    # q^T (+rope) for query tokens of this chunk
                    qs = 128 if s == 0 else 0
                    qL = L - qs
                    qt = 10 if s == 0 else ct  # chunk0 q uses shifted table
                    if qL > 0:
                        for nch in range(8):
                            qp = mps.tile([128, 512], F32, tag="mm")
                            for kc in range(8):
                                nc.tensor.matmul(qp[:, 0:qL],
                                                 w_sb[:, kc, nch * 128:(nch + 1) * 128],
                                                 xT[:, kc, qs:L],
                                                 start=(kc == 0), stop=(kc == 7))
                            qf = rop.tile([128, 512], F32R, tag="ro")
                            _rope(nc, tmpp, qp, qf, 0, qL, rp, qt, qt + 1)
                            q0 = s + qs - 128
                            nc.sync.dma_start(
                                qrope[nch * 128:(nch + 1) * 128, q0:q0 + qL],
                                qf[:, 0:qL])

                    # k^T roped once with global angles
                    for nch in range(8):
                        kp = mps.tile([128, 512], F32, tag="mm")
                        for kc in range(8):
                            nc.tensor.matmul(kp[:, :L],
                                             w_sb[:, kc, 1024 + nch * 128: 1024 + (nch + 1) * 128],
                                             xT[:, kc, 0:L],
                                             start=(kc == 0), stop=(kc == 7))
                        kf = rop.tile([128, 512], F32R, tag="ro")
                        _rope(nc, tmpp, kp, kf, 0, L, rp, ct, ct + 1)
                        nc.sync.dma_start(
                            kT[nch * 128:(nch + 1) * 128, s:s + L], kf[:, :L])

                    # v in natural layout, bf16
                    for mt in range(nmt):
                        vsb = vp.tile([128, D], BF16, tag="v")
                        for nh in range(2):
                            vq = mps.tile([128, 512], F32, tag="mm")
                            for kc in range(8):
                                nc.tensor.matmul(vq[:],
                                                 xT[:, kc, mt * 128:(mt + 1) * 128],
                                                 w_sb[:, kc, 2048 + nh * 512: 2048 + (nh + 1) * 512],
                                                 start=(kc == 0), stop=(kc == 7))
                            nc.scalar.copy(vsb[:, nh * 512:(nh + 1) * 512], vq[:])
                        nc.sync.dma_start(vstage[s + mt * 128: s + (mt + 1) * 128, :], vsb[:])

            # ---------------- Phase C: windowed attention ----------------
            with ExitStack() as cd:
                atp = cd.enter_context(tc.tile_pool(name="aT", bufs=1))
                aT = atp.tile([128, 8, HALF], BF16, tag="aT")
                if True:
                    cc = cd
                    qwp = cc.enter_context(tc.tile_pool(name="qw", bufs=3))
                    k2p = cc.enter_context(tc.tile_pool(name="k2w", bufs=3))
                    vwp = cc.enter_context(tc.tile_pool(name="vw", bufs=4))
                    ep = cc.enter_context(tc.tile_pool(name="e", bufs=4))
                    pp = cc.enter_context(tc.tile_pool(name="p", bufs=3))
                    ptp = cc.enter_context(tc.tile_pool(name="pt", bufs=3))
                    sump = cc.enter_context(tc.tile_pool(name="sums", bufs=4))
                    sps = cc.enter_context(tc.tile_pool(name="sps", bufs=2, space="PSUM"))
                    tps2 = cc.enter_context(tc.tile_pool(name="tps2", bufs=2, space="PSUM"))
                    aps = cc.enter_context(tc.tile_pool(name="aps", bufs=2, space="PSUM"))
                    # phase D pools live alongside C so D(w-1) interleaves
                    # with C(w): D is PE-heavy, C is DVE/ACT-heavy.
                    wop = cc.enter_context(tc.tile_pool(name="wo", bufs=1))
                    wo = wop.tile([128, 8, D], BF16, tag="wo")
                    nc.sync.dma_start(wo[:], wout.rearrange("(c p) n -> p c n", p=128))
                    outp = cc.enter_context(tc.tile_pool(name="outsb", bufs=2))
                    ops = cc.enter_context(tc.tile_pool(name="ops", bufs=2, space="PSUM"))

                    def emit_d(mt):
                        osb = outp.tile([128, D], F32, tag="o")
                        for nh in range(2):
                            op_ = ops.tile([128, 512], F32, tag="op")
                            for kc in range(8):
                                nc.tensor.matmul(op_[:],
                                                 aT[:, kc, mt * 128:(mt + 1) * 128],
                                                 wo[:, kc, nh * 512:(nh + 1) * 512],
                                                 start=(kc == 0), stop=(kc == 7))
                            nc.scalar.copy(osb[:, nh * 512:(nh + 1) * 512], op_[:])
                        nc.sync.dma_start(out[mt * 128:(mt + 1) * 128, :], osb[:])

                    # un-permute staged q/k on load: target row groups
                    # [hA_lo, hA_hi, hB_lo, hB_hi] <- permuted-source offsets
                    UNP = (0, 64, 32, 96)
                    vtiles = {}
                    for w in range(NWIN):
                        qsrc = qrope[:, w * 128:(w + 1) * 128] \
                            .rearrange("(c p) m -> p c m", p=128)
                        qw = qwp.tile([128, 8, 128], F32R, tag="qw")
                        for g, off in enumerate(UNP):
                            nc.sync.dma_start(qw[g * 32:(g + 1) * 32, :, :],
                                              qsrc[off:off + 32])
                        ksrc = kT[:, w * 128: w * 128 + 256] \
                            .rearrange("(c p) j -> p c j", p=128)
                        k2w = k2p.tile([128, 8, 256], F32R, tag="k2w")
                        for g, off in enumerate(UNP):
                            nc.sync.dma_start(k2w[g * 32:(g + 1) * 32, :, :],
                                              ksrc[off:off + 32])
                        for vt in ([w, w + 1] if w == 0 else [w + 1]):
                            v_t = vwp.tile([128, D], BF16, tag="vw")
                            nc.sync.dma_start(v_t[:], vstage[vt * 128:(vt + 1) * 128, :])
                            vtiles[vt] = v_t
                        mslot = 0 if w == 0 else 1
                        for blk in range(8):          # head pair 2*blk, 2*blk+1
                            # scores are bounded (|s|<~8) so exp the raw PSUM
                            # scores; the mask is applied as a binary multiply
                            # fused with the in-band row-sum below.
                            ee = ep.tile([128, 512], F32, tag="ee")
                            for sub in range(2):
                                spx = sps.tile([128, 256], F32, tag="s")
                                nc.tensor.matmul(spx[:], qw[sub * 64:sub * 64 + 64, blk, :],
                                                 k2w[sub * 64:sub * 64 + 64, blk, :],
                                                 start=True, stop=True)
                                nc.scalar.activation(ee[:, sub * 256:(sub + 1) * 256],
                                                     spx[:], EXP)
                            eb = pp.tile([128, 512], BF16, tag="eb")
                            ssum = sump.tile([128, 2], F32, tag="ss")
                            nc.vector.scalar_tensor_tensor(
                                eb[:, 0:256], ee[:, 0:256], 1.0, mk[:, mslot, 0:256],
                                MUL, MUL, accum_out=ssum[:, 0:1])
                            nc.vector.scalar_tensor_tensor(
                                eb[:, 256:512], ee[:, 256:512], 1.0,
                                mk[:, mslot, 256:512],
                                MUL, MUL, accum_out=ssum[:, 1:2])
                            rr = sump.tile([128, 2], F32, tag="rr")
                            nc.vector.reciprocal(rr[:], ssum[:])
                            pf = pp.tile([128, 512], BF16, tag="pf")
                            nc.vector.tensor_scalar_mul(pf[:, 0:256], eb[:, 0:256],
                                                        rr[:, 0:1])
                            nc.vector.tensor_scalar_mul(pf[:, 256:512], eb[:, 256:512],
                                                        rr[:, 1:2])
                            ptq = tps2.tile([128, 512], BF16, tag="ptq")
                            for i in range(4):
                                nc.tensor.transpose(ptq[:, i * 128:(i + 1) * 128],
                                                    pf[:, i * 128:(i + 1) * 128],
                                                    identb[:])
                            pt = ptp.tile([128, 512], BF16, tag="pt")
                            nc.scalar.copy(pt[:], ptq[:])
                            cA, cB = blk * 128, blk * 128 + 64
                            ap_ = aps.tile([64, 256], F32, tag="ap")
                            nc.tensor.matmul(ap_[:, 0:128], vtiles[w][:, cA:cA + 64],
                                             pt[:, 0:128], start=True, stop=False)
                            nc.tensor.matmul(ap_[:, 0:128], vtiles[w + 1][:, cA:cA + 64],
                                             pt[:, 128:256], start=False, stop=True)
                            nc.tensor.matmul(ap_[:, 128:256], vtiles[w][:, cB:cB + 64],
                                             pt[:, 256:384], start=True, stop=False)
                            nc.tensor.matmul(ap_[:, 128:256], vtiles[w + 1][:, cB:cB + 64],
                                             pt[:, 384:512], start=False, stop=True)
                            nc.vector.tensor_copy(
                                aT[0:64, blk, w * 128:(w + 1) * 128], ap_[:, 0:128])
                            nc.vector.tensor_copy(
                                aT[64:128, blk, w * 128:(w + 1) * 128], ap_[:, 128:256])
                        vtiles.pop(w - 1, None)
                    for mt in range(16):
                        emit_d(mt)

    nc.compile()
    return nc


_NC = {}


def _get_nc(reps=1):
    if reps not in _NC:
        _NC[reps] = _build(reps)
    return _NC[reps]


# permutation within each 128-row (2-head) block of head-transposed q/k:
# new row r holds old row ((r//32)%2)*64 + (r%32) + 32*(r//64)
_r = np.arange(128)
_PERM = ((_r // 32) % 2) * 64 + (_r % 32) + 32 * (_r // 64)


def _host_inputs(x, W_qkv, W_out):
    import ml_dtypes
    # permute q and k column blocks of W_qkv; fold the q scale into W so q
    # and k share one global-angle rope table per chunk
    W = np.ascontiguousarray(W_qkv, np.float32).copy()
    for sec in range(2):                     # q section, k section
        for b in range(8):
            base = sec * 1024 + b * 128
            W[:, base:base + 128] = W[:, base + _PERM]
    W[:, 0:1024] *= SCALE

    invf = THETA ** (-(np.arange(0, 64, 2) / 64.0))          # [32]
    rows_f = invf[_r % 32]                                   # [128] freq per row
    # sin tiles are indexed by SOURCE row of the rotate (partner r^64);
    # the destination sign is +1 when the source is a hi-half (r>=64).
    rows_s = np.where(_r < 64, 1.0, -1.0)
    # global angle = freq * xs-row index; q at xs row t and key at xs row t
    # use the same angle, so relative phase matches the reference exactly.
    ropes = np.empty((12, 128, 512), np.float32)
    for ci, (s, _L) in enumerate(CHUNKS):
        ang = rows_f[:, None] * (s + np.arange(512))[None, :]
        ropes[2 * ci] = np.cos(ang)
        ropes[2 * ci + 1] = rows_s[:, None] * np.sin(ang)
    ang0 = rows_f[:, None] * (128 + np.arange(512))[None, :]  # chunk0 q
    ropes[10] = np.cos(ang0)
    ropes[11] = rows_s[:, None] * np.sin(ang0)

    i = np.arange(128)[:, None]
    jj = np.arange(256)[None, :]
    band = (jj >= i) & (jj <= i + 128)
    maskB = np.where(band, 1.0, 0.0).astype(np.float32)          # binary mask
    maskA0 = np.where(band & (jj >= 128), 1.0, 0.0).astype(np.float32)
    maskB = np.concatenate([maskB, maskB], axis=1)
    maskA0 = np.concatenate([maskA0, maskA0], axis=1)

    in_maps = []
    for c in range(NCORES):
        bi, hi = c // 2, c % 2
        xsh = np.empty((NT, D), np.float32)
        if hi == 0:
            xsh[:WS] = 0.0
            xsh[WS:] = x[bi, 0:HALF]
            mA = maskA0
        else:
            xsh[:] = x[bi, HALF - WS: N]
            mA = maskB
        in_maps.append({
            "xs": xsh,
            "wqkv": W,
            "wout": np.ascontiguousarray(W_out, np.float32).astype(ml_dtypes.bfloat16),
            "ropes": ropes,
            "masks": np.stack([mA, maskB]),
        })
    return in_maps


def kernel(x, W_qkv, W_out):
    x = np.asarray(x, np.float32)
    nc = _get_nc()
    in_maps = _host_inputs(x, W_qkv, W_out)
    res = run_bass_kernel_spmd(nc, in_maps, list(range(NCORES)))
    outf = np.empty((B, N, D), np.float32)
    for c in range(NCORES):
        bi, hi = c // 2, c % 2
        outf[bi, hi * HALF:(hi + 1) * HALF] = res.results[c]["out"]
    return outf

